# revision 18
# baseline (speedup 1.0000x reference)
"""Trainium2 Bass kernel for nn_EntityEncoder — batch-parallel, no collectives.

Each core owns 4 batches (64 paths) and reads the FULL vocab for them:
  - x slice [64 paths, 50000] -> transposed/padded [128, 391*64] fp8 (3.2 MB)
  - full [emb | ones] [128, 391*129] bf16 (12.9 MB, SBUF-resident)
One PSUM accumulation [64, 129] over 391 K=128 subtiles gives sums AND
counts (ones column) with no cross-core exchange at all — no ncfw barrier
(~41 us), no ReduceScatter (~11 us). Head is fully local per core.
"""

import numpy as np

B, P, E, H = 32, 16, 50000, 128
NCORES = 8
BP = B * P
EPS = 1e-5
NB = BP // NCORES           # 64 local paths
BL = B // NCORES            # 4 local batches
SUB = 128
E_PAD = 50048
NSUB = E_PAD // SUB         # 391
SPC = 25                    # subtiles per DMA chunk
CHUNKS = [SPC] * 15 + [NSUB - 15 * SPC]   # 15x25 + 16

NPAR = 268  # same packed-params layout as the vocab-sharded kernel

_cached = {}


def _build(lean1=False, lean2=False):
    import concourse.bacc as bacc
    import concourse.mybir as mybir
    import concourse.tile as tile
    from concourse import masks

    f32 = mybir.dt.float32
    bf16 = mybir.dt.bfloat16
    fp8 = mybir.dt.float8e4

    nc = bacc.Bacc("TRN2", target_bir_lowering=False, debug=False,
                   num_devices=NCORES)

    x_d = nc.dram_tensor("x", [SUB, NSUB * NB], fp8, kind="ExternalInput")
    emb_d = nc.dram_tensor("emb", [SUB, NSUB * 129], bf16,
                           kind="ExternalInput")
    par_d = nc.dram_tensor("par", [128, NPAR], f32, kind="ExternalInput")
    out_d = nc.dram_tensor("out", [BL, H], f32, kind="ExternalOutput")

    with tile.TileContext(nc) as tc:
        with tc.tile_pool(name="const", bufs=1) as constp, \
             tc.tile_pool(name="xin", bufs=4) as xin, \
             tc.tile_pool(name="ein", bufs=4) as ein, \
             tc.tile_pool(name="head", bufs=1) as head, \
             tc.tile_pool(name="ps_acc", bufs=1, space="PSUM") as ps_acc, \
             tc.tile_pool(name="ps_head", bufs=4, space="PSUM") as ps_head:

            ident = constp.tile([128, 128], f32)
            masks.make_identity(nc, ident[:])
            par = constp.tile([128, NPAR], f32)
            nc.sync.dma_start(par[:], par_d[:, :])

            warm = constp.tile([1, 1], f32)
            nc.scalar.activation(warm[:], par[0:1, 7:8],
                                 mybir.ActivationFunctionType.Sqrt,
                                 bias=par[0:1, 7:8], scale=1.0)

            ps0 = ps_acc.tile([NB, 512], f32, name="acc")
            goff = 0
            for t, S in enumerate(CHUNKS):
                xt = xin.tile([SUB, S * NB], fp8, tag="xt", name=f"xt{t}")
                nc.gpsimd.dma_start(
                    xt[:], x_d[:, goff * NB:(goff + S) * NB])
                et = ein.tile([SUB, S * 129], bf16, tag="et", name=f"et{t}")
                # alternate emb chunks across both HWDGE queues for bandwidth
                eng = nc.scalar if t % 2 == 0 else nc.sync
                eng.dma_start(
                    et[:], emb_d[:, goff * 129:(goff + S) * 129])
                for j in range(S):
                    g = goff + j
                    nc.tensor.matmul(
                        ps0[:, 0:129],
                        xt[:, j * NB:(j + 1) * NB],
                        et[:, j * 129:(j + 1) * 129],
                        start=(g == 0), stop=(g == NSUB - 1))
                goff += S

            # ---- head on local [64, 129] totals ----
            Ssb = head.tile([NB, 129], f32)
            nc.vector.tensor_copy(Ssb[:], ps0[:, 0:129])

            rec = head.tile([NB, 1], f32)
            nc.vector.reciprocal(rec[:], Ssb[:, 128:129])
            R = head.tile([NB, BL], f32)
            nc.vector.tensor_scalar(
                out=R[:], in0=par[0:NB, 8:12], scalar1=rec[:, 0:1],
                scalar2=None, op0=mybir.AluOpType.mult)

            x0_ps = ps_head.tile([BL, 128], f32, tag="psh", name="x0")
            nc.tensor.matmul(x0_ps[:], R[:], Ssb[:, 0:128],
                             start=True, stop=True)

            def layer_norm(x_ps, eps_col, name):
                st6 = head.tile([BL, 6], f32, tag=f"{name}_st6")
                nc.vector.bn_stats(st6[:], x_ps[:])
                mv = head.tile([BL, 2], f32, tag=f"{name}_mv")
                nc.vector.bn_aggr(mv[:], st6[:])
                sd = head.tile([BL, 1], f32, tag=f"{name}_sd")
                nc.scalar.activation(sd[:], mv[:, 1:2],
                                     mybir.ActivationFunctionType.Sqrt,
                                     bias=par[0:BL, eps_col:eps_col + 1],
                                     scale=1.0)
                rstd = head.tile([BL, 1], f32, tag=f"{name}_rstd")
                nc.vector.reciprocal(rstd[:], sd[:])
                xn = head.tile([BL, 128], f32, tag=f"{name}_xn")
                nc.vector.tensor_scalar(
                    out=xn[:], in0=x_ps[:],
                    scalar1=mv[:, 0:1], scalar2=rstd[:, 0:1],
                    op0=mybir.AluOpType.subtract, op1=mybir.AluOpType.mult)
                return xn

            def linear_relu_bn(xn, w_lo, b_col, bng_col, bnb_col, lean,
                               name):
                xt_ps = ps_head.tile([128, BL], f32, tag="psh",
                                     name=f"{name}_xt")
                nc.tensor.transpose(xt_ps[:], xn[:], ident[0:BL, 0:BL])
                xt_sb = head.tile([128, BL], f32, tag=f"{name}_xts")
                nc.vector.tensor_copy(xt_sb[:], xt_ps[:])
                y_ps = ps_head.tile([128, BL], f32, tag="psh",
                                    name=f"{name}_y")
                nc.tensor.matmul(y_ps[:], par[:, w_lo:w_lo + 128], xt_sb[:],
                                 start=True, stop=True)
                y = head.tile([128, BL], f32, tag=f"{name}_relu")
                nc.vector.tensor_scalar(
                    out=y[:], in0=y_ps[:],
                    scalar1=par[:, b_col:b_col + 1], scalar2=0.0,
                    op0=mybir.AluOpType.add, op1=mybir.AluOpType.max)
                if lean:
                    return y
                z = head.tile([128, BL], f32, tag=f"{name}_bn")
                nc.vector.tensor_scalar(
                    out=z[:], in0=y[:],
                    scalar1=par[:, bng_col:bng_col + 1],
                    scalar2=par[:, bnb_col:bnb_col + 1],
                    op0=mybir.AluOpType.mult, op1=mybir.AluOpType.add)
                return z

            h1 = layer_norm(x0_ps, 6, "ln1")
            z1 = linear_relu_bn(h1, 12, 0, 2, 3, lean1, "l1")
            z1t_ps = ps_head.tile([BL, 128], f32, tag="psh", name="z1t")
            nc.tensor.transpose(z1t_ps[:], z1[:], ident[:, :])
            h2 = layer_norm(z1t_ps, 7, "ln2")
            z2 = linear_relu_bn(h2, 140, 1, 4, 5, lean2, "l2")

            out_ps = ps_head.tile([BL, 128], f32, tag="psh", name="outT")
            nc.tensor.transpose(out_ps[:], z2[:], ident[:, :])
            out_sb = head.tile([BL, 128], f32)
            nc.vector.tensor_copy(out_sb[:], out_ps[:])
            nc.scalar.dma_start(out_d[:, :], out_sb[:])

    nc.compile()
    return nc


def _prepare_in_maps(inputs):
    import ml_dtypes

    x = np.asarray(inputs["inputs"])
    emb = np.asarray(inputs["emb"], dtype=np.float32)
    w1 = np.asarray(inputs["w1"], dtype=np.float32)
    b1 = np.asarray(inputs["b1"], dtype=np.float32)
    w2 = np.asarray(inputs["w2"], dtype=np.float32)
    b2 = np.asarray(inputs["b2"], dtype=np.float32)
    ln1_g = np.asarray(inputs["ln1_g"], np.float32)
    ln1_b = np.asarray(inputs["ln1_b"], np.float32)
    ln2_g = np.asarray(inputs["ln2_g"], np.float32)
    ln2_b = np.asarray(inputs["ln2_b"], np.float32)

    par = np.zeros((128, NPAR), dtype=np.float32)
    w1f = w1 * ln1_g[None, :]
    b1f = b1 + w1 @ ln1_b
    w2f = w2 * ln2_g[None, :]
    b2f = b2 + w2 @ ln2_b
    bn1_g = np.asarray(inputs["bn1_g"], np.float32) / np.sqrt(
        np.float32(1.0) + np.float32(EPS))
    bn1_b = np.asarray(inputs["bn1_b"], np.float32)
    bn2_g = np.asarray(inputs["bn2_g"], np.float32) / np.sqrt(
        np.float32(1.0) + np.float32(EPS))
    bn2_b = np.asarray(inputs["bn2_b"], np.float32)
    lean1 = bool((bn1_g > 0).all() and (bn1_b == 0).all())
    lean2 = bool((bn2_g > 0).all() and (bn2_b == 0).all())
    if lean1:
        w1f = w1f * bn1_g[:, None]
        b1f = b1f * bn1_g
    if lean2:
        w2f = w2f * bn2_g[:, None]
        b2f = b2f * bn2_g
    par[:, 0] = b1f
    par[:, 1] = b2f
    par[:, 2] = bn1_g
    par[:, 3] = bn1_b
    par[:, 4] = bn2_g
    par[:, 5] = bn2_b
    par[:, 6] = EPS * P * P
    par[:, 7] = EPS
    for i in range(NB):
        par[i, 8 + i // P] = 1.0
    par[:, 12:140] = w1f.T
    par[:, 140:268] = w2f.T

    # shared [emb | ones] in subtile-major bf16, built once
    seg_e = np.zeros((E_PAD, 129), dtype=np.float32)
    seg_e[:E, 0:128] = emb
    seg_e[0, 0:128] = 0.0       # padding_idx=0
    seg_e[:, 128] = 1.0
    emb_sh = np.ascontiguousarray(
        seg_e.reshape(NSUB, SUB, 129).transpose(1, 0, 2)
    ).reshape(SUB, NSUB * 129).astype(ml_dtypes.bfloat16)

    x_flat = np.asarray(x).reshape(BP, E)
    in_maps = []
    for c in range(NCORES):
        seg_t = np.zeros((E_PAD, NB), dtype=np.int8)
        seg_t[:E] = (x_flat[c * NB:(c + 1) * NB, :].T == 1)
        x_sh = np.ascontiguousarray(
            seg_t.reshape(NSUB, SUB, NB).transpose(1, 0, 2)
        ).reshape(SUB, NSUB * NB).astype(ml_dtypes.float8_e4m3)
        in_maps.append({"x": x_sh, "emb": emb_sh, "par": par})
    return in_maps, lean1, lean2


def _run(inputs, trace=False):
    from concourse.bass_utils import run_bass_kernel_spmd

    in_maps, lean1, lean2 = _prepare_in_maps(inputs)
    key = ("nc", lean1, lean2)
    if key not in _cached:
        _cached[key] = _build(lean1, lean2)
    nc = _cached[key]
    res = run_bass_kernel_spmd(
        nc, in_maps, core_ids=list(range(NCORES)), trace=trace)
    out = np.concatenate(
        [np.asarray(res.results[c]["out"]) for c in range(NCORES)], axis=0)
    return out, res.exec_time_ns


def kernel(**inputs) -> np.ndarray:
    out, _ = _run(inputs, trace=False)
    return out
